# revision 11
# baseline (speedup 1.0000x reference)
"""3-layer GraphSAGE (ClusterGCN-style) on 8 Trainium2 NeuronCores.

Strategy (graph/data parallel, transform-first):
  - Nodes sharded by contiguous range across 8 cores (6250 each).
  - Per layer l: t = h @ Wl computed on own shard -> AllGather t (full
    node table in each core's DRAM) -> per 128-dst block: dma_gather the
    incoming edges' t[src] rows -> segment-sum via one-hot matmul on the
    tensor engine (S built on-device from dst-local ids; deg_inv folded
    into S) accumulated in PSUM together with the root path h @ Wr and
    the bias (ones-vector matmul) -> relu (+cast fp16) on ScalarE ->
    transpose back to feature-major (PE transposes) for the next layer's
    matmuls.
  - Edges are dst-sorted on host, split per (block, class) where class =
    src < 32768 (dma_gather indices are int16), padded to 128-edge
    chunks with a static chunk count (max over cores -> one SPMD
    program).
  - fp16 data path, fp32 PSUM accumulation, fp32 output. Final layer
    tables are fp32 (64-wide rows must be 256B-aligned for dma_gather);
    gathered fp32 messages are cast to fp16 before the one-hot matmuls.
"""

import math
import numpy as np

N_NODES = 50000
N_EDGES = 800000
D_IN = 512
D_HID = 512
D_OUT = 64
N_CORES = 8
LOW_LIM_FULL = 32768


# ---------------------------------------------------------------------------
# Host preprocessing
# ---------------------------------------------------------------------------

class Plan:
    pass


def _wrap_idx(v):
    """Pack an index vector (len multiple of 16) into the [16, m/16]
    pattern dma_gather expects, replicated to 128 partitions."""
    a = np.asarray(v, np.int16).reshape(-1, 16).T  # [16, m/16]
    return np.tile(a, (8, 1))  # [128, m/16]


def preprocess(x, edge_index, n_nodes, n_cores, d_in, low_lim):
    """Returns (plan, per_core_inputs_list)."""
    src = np.asarray(edge_index[0], np.int64)
    dst = np.asarray(edge_index[1], np.int64)
    nsh = n_nodes // n_cores
    nblk = math.ceil(nsh / 128)
    nfree = nblk * 128
    kc = d_in // 128

    deg = np.bincount(dst, minlength=n_nodes).astype(np.float32)
    deginv = (1.0 / np.maximum(deg, 1.0)).astype(np.float32)

    core = dst // nsh
    bid = core * nblk + (dst - core * nsh) // 128
    cls = (src >= low_lim).astype(np.int64)
    order = np.lexsort((dst, cls, bid))
    src_s = src[order]
    dst_s = dst[order]
    bid_s = bid[order]
    cls_s = cls[order]

    key = bid_s * 2 + cls_s
    ngrp = n_cores * nblk * 2
    starts = np.searchsorted(key, np.arange(ngrp + 1), side="left")

    # static chunk counts per (block, class): max over cores
    cnt = (starts[1:] - starts[:-1]).reshape(n_cores, nblk, 2)
    nchunk = -(-cnt // 128)  # ceil
    nL = nchunk[:, :, 0].max(axis=0)  # [nblk]
    nH = nchunk[:, :, 1].max(axis=0)
    CL = int(nL.sum())
    CH = int(nH.sum())
    TOTC = CL + CH
    offL = np.concatenate([[0], np.cumsum(nL)[:-1]]).astype(np.int64)
    offH = np.concatenate([[0], np.cumsum(nH)[:-1]]).astype(np.int64)
    offT = np.concatenate([[0], np.cumsum(nL + nH)[:-1]]).astype(np.int64)

    plan = Plan()
    plan.nsh, plan.nblk, plan.nfree, plan.kc = nsh, nblk, nfree, kc
    plan.nL, plan.nH = nL.tolist(), nH.tolist()
    plan.CL, plan.CH, plan.TOTC = CL, CH, TOTC
    plan.offL, plan.offH, plan.offT = offL.tolist(), offH.tolist(), offT.tolist()
    plan.low_lim = low_lim
    plan.n_cores = n_cores
    plan.n_nodes = n_nodes

    x = np.asarray(x, np.float32)
    per_core = []
    for c in range(n_cores):
        idxL = np.zeros((CL * 128,), np.int64)
        idxH = np.zeros((CH * 128,), np.int64)
        dloc = np.full((TOTC * 128,), -1.0, np.float32)
        for b in range(nblk):
            for t, (nX, offX, idxX, base) in enumerate(
                ((nL[b], offL[b], idxL, 0), (nH[b], offH[b], idxH, low_lim))
            ):
                g = (c * nblk + b) * 2 + t
                s0, s1 = starts[g], starts[g + 1]
                n_e = s1 - s0
                if nX == 0:
                    continue
                seg = idxX[offX * 128:(offX + nX) * 128]
                seg[:n_e] = src_s[s0:s1] - base
                # dloc columns: block-major, L chunks then H chunks
                dof = (offT[b] + (0 if t == 0 else nL[b])) * 128
                dseg = dloc[dof:dof + nX * 128]
                dseg[:n_e] = (dst_s[s0:s1] - c * nsh - b * 128).astype(np.float32)
        wi_L = _wrap_idx(idxL) if CL else np.zeros((128, 0), np.int16)
        wi_H = _wrap_idx(idxH) if CH else np.zeros((128, 0), np.int16)
        # dloc layout must match gather output: edge i -> partition i%128,
        # chunk i//128; dstloc[p, chunk] = dloc of that edge
        dl = dloc.reshape(TOTC, 128).T.astype(np.float16)  # [128, TOTC]

        dg = np.zeros((nfree,), np.float32)
        dg[:nsh] = deginv[c * nsh:(c + 1) * nsh]
        dg_b = np.broadcast_to(dg.astype(np.float16), (128, nfree)).copy()

        xT = np.zeros((kc, 128, nfree), np.float16)
        xs = x[c * nsh:(c + 1) * nsh]  # [nsh, d_in]
        xT[:, :, :nsh] = xs.T.reshape(kc, 128, nsh)

        per_core.append({
            "xT": xT,
            "idxL": np.ascontiguousarray(wi_L),
            "idxH": np.ascontiguousarray(wi_H),
            "dstloc": np.ascontiguousarray(dl),
            "deginv": dg_b,
        })
    return plan, per_core


# ---------------------------------------------------------------------------
# Device program
# ---------------------------------------------------------------------------

def build_program(plan, d_in, d_hid, d_out):
    import os
    dbg = set(os.environ.get("KDBG", "").split(",")) - {""}
    import concourse.bacc as bacc
    import concourse.tile as tile
    from concourse import bass, mybir
    from concourse.masks import make_identity

    f16 = mybir.dt.float16
    f32 = mybir.dt.float32
    i16 = mybir.dt.int16

    nsh, nblk, nfree = plan.nsh, plan.nblk, plan.nfree
    kcs = [d_in // 128, d_hid // 128, d_hid // 128]
    douts = [d_hid, d_hid, d_out]
    n_cores = plan.n_cores
    n_nodes = plan.n_nodes
    CL, CH, TOTC = plan.CL, plan.CH, plan.TOTC
    low_lim = plan.low_lim
    kc0 = kcs[0]

    nc = bacc.Bacc(
        "TRN2",
        target_bir_lowering=False,
        debug=False,
        num_devices=n_cores,
    )

    xT_d = nc.dram_tensor("xT", [kc0, 128, nfree], f16, kind="ExternalInput").ap()
    w_d = {}
    for l in range(3):
        kd = [d_in, d_hid, d_hid][l]
        w_d[(l, "l")] = nc.dram_tensor(f"wl{l}", [kd, douts[l]], f16,
                                       kind="ExternalInput").ap()
        w_d[(l, "r")] = nc.dram_tensor(f"wr{l}", [kd, douts[l]], f16,
                                       kind="ExternalInput").ap()
    b_d = [nc.dram_tensor(f"b{l}", [1, douts[l]], f16, kind="ExternalInput").ap()
           for l in range(3)]
    idxL_d = nc.dram_tensor("idxL", [128, max(CL * 8, 1)], i16,
                            kind="ExternalInput").ap()
    idxH_d = nc.dram_tensor("idxH", [128, max(CH * 8, 1)], i16,
                            kind="ExternalInput").ap()
    dstloc_d = nc.dram_tensor("dstloc", [128, TOTC], f16, kind="ExternalInput").ap()
    deginv_d = nc.dram_tensor("deginv", [128, nfree], f16, kind="ExternalInput").ap()
    out_d = nc.dram_tensor("out", [nsh, d_out], f32, kind="ExternalOutput").ap()

    with tile.TileContext(nc) as tc:
        # --- DRAM bounce buffers for the AllGathers
        ag_in, ag_out = [], []
        for l in range(3):
            tdt = f16 if l < 2 else f32
            ai = nc.dram_tensor(f"agi{l}", [nsh, douts[l]], tdt,
                                kind="Internal").ap()
            ao = nc.dram_tensor(f"ago{l}", [n_nodes, douts[l]], tdt,
                                kind="Internal", addr_space="Shared").ap()
            ag_in.append(ai)
            ag_out.append(ao)

        import contextlib
        with contextlib.ExitStack() as ctx:
            nb = 1 if "serial" in dbg else 2
            cpool = ctx.enter_context(tc.tile_pool(name="const", bufs=1))
            pt_pool = ctx.enter_context(
                tc.tile_pool(name="pt", bufs=nb, space="PSUM"))
            pm_pool = ctx.enter_context(
                tc.tile_pool(name="pm", bufs=nb, space="PSUM"))
            tr_pool = ctx.enter_context(
                tc.tile_pool(name="tr", bufs=nb, space="PSUM"))
            tsb_pool = ctx.enter_context(tc.tile_pool(name="tsb", bufs=nb))
            msgL_pool = ctx.enter_context(tc.tile_pool(name="msgL", bufs=nb))
            msgH_pool = ctx.enter_context(tc.tile_pool(name="msgH", bufs=nb))
            m16_pool = ctx.enter_context(tc.tile_pool(name="m16", bufs=nb))
            s_pool = ctx.enter_context(tc.tile_pool(name="spool", bufs=nb))
            h_pool = ctx.enter_context(tc.tile_pool(name="hpool", bufs=nb))
            o_pool = ctx.enter_context(tc.tile_pool(name="opool", bufs=nb))

            # --- constants
            hT = cpool.tile([128, kc0 * nfree], f16, name="hT")
            hT3 = hT[:].rearrange("p (q n) -> p q n", n=nfree)
            nc.sync.dma_start(hT3, xT_d.rearrange("q p n -> p q n"))
            if "noping" not in dbg:
                hTb = cpool.tile([128, kc0 * nfree], f16, name="hTb")
                hT3b = hTb[:].rearrange("p (q n) -> p q n", n=nfree)
                hts = [hT3, hT3b]
            else:
                hts = [hT3, hT3]

            ident = cpool.tile([128, 128], f16, name="ident")
            make_identity(nc, ident[:])
            iota = cpool.tile([128, 128], f16, name="iota")
            nc.gpsimd.iota(iota[:], pattern=[[1, 128]], base=0,
                           channel_multiplier=0,
                           allow_small_or_imprecise_dtypes=True)
            ones = cpool.tile([1, 128], f16, name="ones")
            nc.vector.memset(ones[:], 1.0)

            wt = {}
            for l in range(3):
                kd = kcs[l]
                for s in ("l", "r"):
                    t = cpool.tile([128, kd * douts[l]], f16, name=f"w{s}{l}")
                    nc.sync.dma_start(
                        t[:].rearrange("p (q d) -> p q d", d=douts[l]),
                        w_d[(l, s)].rearrange("(q p) d -> p q d", p=128))
                    wt[(l, s)] = t
            bt = []
            for l in range(3):
                t = cpool.tile([1, douts[l]], f16, name=f"bt{l}")
                nc.sync.dma_start(t[:], b_d[l][:, :])
                bt.append(t)

            idxL_t = cpool.tile([128, max(CL * 8, 1)], i16, name="idxLt")
            nc.sync.dma_start(idxL_t[:], idxL_d[:, :])
            idxH_t = cpool.tile([128, max(CH * 8, 1)], i16, name="idxHt")
            nc.sync.dma_start(idxH_t[:], idxH_d[:, :])
            dstloc_t = cpool.tile([128, TOTC], f16, name="dstloct")
            nc.sync.dma_start(dstloc_t[:], dstloc_d[:, :])
            deginv_t = cpool.tile([128, nfree], f16, name="deginvt")
            nc.sync.dma_start(deginv_t[:], deginv_d[:, :])

            rg = [list(range(n_cores))]

            for l in range(3):
                dout = douts[l]
                kc = kcs[l]
                tdt = f16 if l < 2 else f32
                hT3 = hts[l % 2]
                hT3n = hts[(l + 1) % 2]

                # ---- phase 1: t = h @ Wl -> ag_in
                for b in range(nblk):
                    bs = slice(b * 128, (b + 1) * 128)
                    rows = min(128, nsh - b * 128)
                    pt = pt_pool.tile([128, dout], f32, tag="pt")
                    for q in range(kc):
                        nc.tensor.matmul(
                            pt[:], lhsT=hT3[:, q, bs],
                            rhs=wt[(l, "l")][:, q * dout:(q + 1) * dout],
                            start=(q == 0), stop=(q == kc - 1))
                    tsb = tsb_pool.tile([128, dout], tdt, tag="tsb")
                    nc.scalar.copy(tsb[:], pt[:])
                    nc.sync.dma_start(ag_in[l][b * 128:b * 128 + rows, :],
                                      tsb[:rows, :])

                # ---- phase 2: AllGather t
                from concourse import mybir as _mb
                if "nocoll" not in dbg:
                    nc.gpsimd.collective_compute(
                        "AllGather", _mb.AluOpType.bypass, replica_groups=rg,
                        ins=[ag_in[l].opt()], outs=[ag_out[l].opt()])

                # ---- phase 3: aggregate + root + combine per block
                for b in range(nblk):
                    bs = slice(b * 128, (b + 1) * 128)
                    rows = min(128, nsh - b * 128)
                    nL, nH = plan.nL[b], plan.nH[b]
                    nT = nL + nH

                    msgL = msgH = None
                    if nL:
                        msgL = msgL_pool.tile([128, nL * dout], tdt, tag="msgL")
                        if "nogather" in dbg:
                            nc.vector.memset(msgL[:], 0.25)
                        else:
                            nc.gpsimd.dma_gather(
                                msgL[:].rearrange("p (c e) -> p c e", e=dout),
                                ag_out[l][:, :],
                                idxL_t[:, plan.offL[b] * 8:(plan.offL[b] + nL) * 8],
                                num_idxs=nL * 128, num_idxs_reg=nL * 128,
                                elem_size=dout, single_packet=False)
                    if nH:
                        msgH = msgH_pool.tile([128, nH * dout], tdt, tag="msgH")
                        if "nogather" in dbg:
                            nc.vector.memset(msgH[:], 0.25)
                        else:
                            nc.gpsimd.dma_gather(
                                msgH[:].rearrange("p (c e) -> p c e", e=dout),
                                ag_out[l][low_lim:, :],
                                idxH_t[:, plan.offH[b] * 8:(plan.offH[b] + nH) * 8],
                                num_idxs=nH * 128, num_idxs_reg=nH * 128,
                                elem_size=dout, single_packet=False)

                    if "noagg" in dbg:
                        nT = nL = nH = 0
                    if nT:
                        S = s_pool.tile([128, nT * 128], f16, tag="S")
                        S3 = S[:].rearrange("p (c d) -> p c d", d=128)
                        dl3 = (dstloc_t[:, plan.offT[b]:plan.offT[b] + nT]
                               .rearrange("p (c o) -> p c o", o=1)
                               .to_broadcast([128, nT, 128]))
                        io3 = (iota[:].rearrange("p (o d) -> p o d", o=1)
                               .to_broadcast([128, nT, 128]))
                        nc.vector.tensor_tensor(
                            out=S3, in0=dl3, in1=io3,
                            op=_mb.AluOpType.is_equal)
                        dg3 = (deginv_t[:, bs]
                               .rearrange("p (o d) -> p o d", o=1)
                               .to_broadcast([128, nT, 128]))
                        nc.vector.tensor_tensor(
                            out=S3, in0=S3, in1=dg3, op=_mb.AluOpType.mult)

                    if l == 2 and nT:
                        m16 = m16_pool.tile([128, nT * dout], f16, tag="m16")
                        if nL:
                            nc.vector.tensor_copy(m16[:, :nL * dout], msgL[:])
                        if nH:
                            nc.vector.tensor_copy(m16[:, nL * dout:], msgH[:])

                    pm = pm_pool.tile([128, dout], f32, tag="pm")
                    for q in range(kc):
                        nc.tensor.matmul(
                            pm[:], lhsT=hT3[:, q, bs],
                            rhs=wt[(l, "r")][:, q * dout:(q + 1) * dout],
                            start=(q == 0), stop=False)
                    nc.tensor.matmul(pm[:], lhsT=ones[:1, :], rhs=bt[l][:1, :],
                                     start=False, stop=(nT == 0))
                    for j in range(nT):
                        if l == 2:
                            rhs = m16[:, j * dout:(j + 1) * dout]
                        elif j < nL:
                            rhs = msgL[:, j * dout:(j + 1) * dout]
                        else:
                            rhs = msgH[:, (j - nL) * dout:(j - nL + 1) * dout]
                        nc.tensor.matmul(pm[:], lhsT=S[:, j * 128:(j + 1) * 128],
                                         rhs=rhs, start=False,
                                         stop=(j == nT - 1))

                    if l < 2:
                        hsb = h_pool.tile([128, dout], f16, tag="h")
                        nc.scalar.activation(
                            hsb[:], pm[:],
                            _mb.ActivationFunctionType.Relu)
                        if "notr" not in dbg:
                            for q in range(kc):
                                ptr = tr_pool.tile([128, 128], f16, tag="tr")
                                nc.tensor.transpose(ptr[:], hsb[:, q * 128:(q + 1) * 128],
                                                    ident[:])
                                nc.vector.tensor_copy(hT3n[:, q, bs], ptr[:])
                    else:
                        osb = o_pool.tile([128, dout], f32, tag="o")
                        nc.scalar.copy(osb[:], pm[:])
                        nc.sync.dma_start(out_d[b * 128:b * 128 + rows, :],
                                          osb[:rows, :])

    nc.compile()
    return nc


# ---------------------------------------------------------------------------
# Entry point
# ---------------------------------------------------------------------------

LAST_RESULTS = None
_CACHE = {}


def _run(x, edge_index, weights, n_nodes, n_cores, d_in, d_hid, d_out,
         low_lim, trace=False):
    global LAST_RESULTS
    from concourse.bass_utils import run_bass_kernel_spmd

    plan, per_core = preprocess(x, edge_index, n_nodes, n_cores, d_in, low_lim)
    fp = (n_nodes, d_in, d_hid, d_out, tuple(plan.nL), tuple(plan.nH))
    if fp not in _CACHE:
        _CACHE[fp] = build_program(plan, d_in, d_hid, d_out)
    nc = _CACHE[fp]

    const = {}
    for l, (Wl, Wr, b) in enumerate(weights):
        const[f"wl{l}"] = np.asarray(Wl, np.float32).astype(np.float16)
        const[f"wr{l}"] = np.asarray(Wr, np.float32).astype(np.float16)
        const[f"b{l}"] = np.asarray(b, np.float32).astype(np.float16)[None, :]

    in_maps = []
    for c in range(n_cores):
        m = dict(const)
        pc = per_core[c]
        m["xT"] = pc["xT"]
        m["idxL"] = pc["idxL"] if plan.CL else np.zeros((128, 1), np.int16)
        m["idxH"] = pc["idxH"] if plan.CH else np.zeros((128, 1), np.int16)
        m["dstloc"] = pc["dstloc"]
        m["deginv"] = pc["deginv"]
        in_maps.append(m)

    res = run_bass_kernel_spmd(nc, in_maps, core_ids=list(range(n_cores)),
                               trace=trace)
    LAST_RESULTS = res
    out = np.concatenate([res.results[c]["out"] for c in range(n_cores)], axis=0)
    return out.astype(np.float32)


def kernel(x, edge_index, relations=None, Wl0=None, Wr0=None, b0=None,
           Wl1=None, Wr1=None, b1=None, Wl2=None, Wr2=None, b2=None,
           **kw):
    x = np.asarray(x, np.float32)
    edge_index = np.asarray(edge_index)
    weights = [(Wl0, Wr0, b0), (Wl1, Wr1, b1), (Wl2, Wr2, b2)]
    import os
    trace = bool(int(os.environ.get("KERNEL_TRACE", "0")))
    return _run(x, edge_index, weights, N_NODES, N_CORES, D_IN, D_HID, D_OUT,
                LOW_LIM_FULL, trace=trace)


# revision 14
# speedup vs baseline: 41.9157x; 41.9157x over previous
"""3-layer GraphSAGE (ClusterGCN-style) on 8 Trainium2 NeuronCores.

Strategy (graph/data parallel, transform-first):
  - Nodes sharded by contiguous range across 8 cores (6250 each).
  - Per layer l: t = h @ Wl computed on own shard -> AllGather t (full
    node table in each core's DRAM) -> per 128-dst block: dma_gather the
    incoming edges' t[src] rows -> segment-sum via one-hot matmul on the
    tensor engine (S built on-device from dst-local ids; deg_inv folded
    into S) accumulated in PSUM together with the root path h @ Wr and
    the bias (ones-vector matmul) -> relu (+cast fp16) on ScalarE ->
    transpose back to feature-major (PE transposes) for the next layer's
    matmuls.
  - Edges are dst-sorted on host, split per (block, class) where class =
    src < 32768 (dma_gather indices are int16), padded to 128-edge
    chunks with a static chunk count (max over cores -> one SPMD
    program).
  - fp16 data path, fp32 PSUM accumulation, fp32 output. Final layer
    tables are fp32 (64-wide rows must be 256B-aligned for dma_gather);
    gathered fp32 messages are cast to fp16 before the one-hot matmuls.
"""

import math
import numpy as np

N_NODES = 50000
N_EDGES = 800000
D_IN = 512
D_HID = 512
D_OUT = 64
N_CORES = 8
LOW_LIM_FULL = 32768


# ---------------------------------------------------------------------------
# Host preprocessing
# ---------------------------------------------------------------------------

class Plan:
    pass


def _wrap_idx(v):
    """Pack an index vector (len multiple of 16) into the [16, m/16]
    pattern dma_gather expects, replicated to 128 partitions."""
    a = np.asarray(v, np.int16).reshape(-1, 16).T  # [16, m/16]
    return np.tile(a, (8, 1))  # [128, m/16]


def preprocess(x, edge_index, n_nodes, n_cores, d_in, low_lim):
    """Returns (plan, per_core_inputs_list)."""
    src = np.asarray(edge_index[0], np.int64)
    dst = np.asarray(edge_index[1], np.int64)
    nsh = n_nodes // n_cores
    nblk = math.ceil(nsh / 128)
    nfree = nblk * 128
    kc = d_in // 128

    deg = np.bincount(dst, minlength=n_nodes).astype(np.float32)
    deginv = (1.0 / np.maximum(deg, 1.0)).astype(np.float32)

    core = dst // nsh
    bid = core * nblk + (dst - core * nsh) // 128
    cls = (src >= low_lim).astype(np.int64)
    order = np.lexsort((dst, cls, bid))
    src_s = src[order]
    dst_s = dst[order]
    bid_s = bid[order]
    cls_s = cls[order]

    key = bid_s * 2 + cls_s
    ngrp = n_cores * nblk * 2
    starts = np.searchsorted(key, np.arange(ngrp + 1), side="left")

    # static chunk counts per (block, class): max over cores
    cnt = (starts[1:] - starts[:-1]).reshape(n_cores, nblk, 2)
    nchunk = -(-cnt // 128)  # ceil
    nL = nchunk[:, :, 0].max(axis=0)  # [nblk]
    nH = nchunk[:, :, 1].max(axis=0)
    CL = int(nL.sum())
    CH = int(nH.sum())
    TOTC = CL + CH
    offL = np.concatenate([[0], np.cumsum(nL)[:-1]]).astype(np.int64)
    offH = np.concatenate([[0], np.cumsum(nH)[:-1]]).astype(np.int64)
    offT = np.concatenate([[0], np.cumsum(nL + nH)[:-1]]).astype(np.int64)

    plan = Plan()
    plan.nsh, plan.nblk, plan.nfree, plan.kc = nsh, nblk, nfree, kc
    plan.nL, plan.nH = nL.tolist(), nH.tolist()
    plan.CL, plan.CH, plan.TOTC = CL, CH, TOTC
    plan.offL, plan.offH, plan.offT = offL.tolist(), offH.tolist(), offT.tolist()
    plan.low_lim = low_lim
    plan.n_cores = n_cores
    plan.n_nodes = n_nodes

    x = np.asarray(x, np.float32)
    per_core = []
    for c in range(n_cores):
        idxL = np.zeros((CL * 128,), np.int64)
        idxH = np.zeros((CH * 128,), np.int64)
        dloc = np.full((TOTC * 128,), -1.0, np.float32)
        for b in range(nblk):
            for t, (nX, offX, idxX, base) in enumerate(
                ((nL[b], offL[b], idxL, 0), (nH[b], offH[b], idxH, low_lim))
            ):
                g = (c * nblk + b) * 2 + t
                s0, s1 = starts[g], starts[g + 1]
                n_e = s1 - s0
                if nX == 0:
                    continue
                seg = idxX[offX * 128:(offX + nX) * 128]
                seg[:n_e] = src_s[s0:s1] - base
                # dloc columns: block-major, L chunks then H chunks
                dof = (offT[b] + (0 if t == 0 else nL[b])) * 128
                dseg = dloc[dof:dof + nX * 128]
                dseg[:n_e] = (dst_s[s0:s1] - c * nsh - b * 128).astype(np.float32)
        wi_L = _wrap_idx(idxL) if CL else np.zeros((128, 0), np.int16)
        wi_H = _wrap_idx(idxH) if CH else np.zeros((128, 0), np.int16)
        # dloc layout must match gather output: edge i -> partition i%128,
        # chunk i//128; dstloc[p, chunk] = dloc of that edge
        dl = dloc.reshape(TOTC, 128).T.astype(np.float16)  # [128, TOTC]

        dg = np.zeros((nfree,), np.float32)
        dg[:nsh] = deginv[c * nsh:(c + 1) * nsh]
        dg_b = np.broadcast_to(dg.astype(np.float16), (128, nfree)).copy()

        xT = np.zeros((kc, 128, nfree), np.float16)
        xs = x[c * nsh:(c + 1) * nsh]  # [nsh, d_in]
        xT[:, :, :nsh] = xs.T.reshape(kc, 128, nsh)

        per_core.append({
            "xT": xT,
            "idxL": np.ascontiguousarray(wi_L),
            "idxH": np.ascontiguousarray(wi_H),
            "dstloc": np.ascontiguousarray(dl),
            "deginv": dg_b,
        })
    return plan, per_core


# ---------------------------------------------------------------------------
# Device program
# ---------------------------------------------------------------------------

def build_program(plan, d_in, d_hid, d_out):
    import os
    dbg = set(os.environ.get("KDBG", "").split(",")) - {""}
    import concourse.bacc as bacc
    import concourse.tile as tile
    from concourse import bass, mybir
    from concourse.masks import make_identity

    f16 = mybir.dt.float16
    f32 = mybir.dt.float32
    i16 = mybir.dt.int16

    nsh, nblk, nfree = plan.nsh, plan.nblk, plan.nfree
    kcs = [d_in // 128, d_hid // 128, d_hid // 128]
    douts = [d_hid, d_hid, d_out]
    n_cores = plan.n_cores
    n_nodes = plan.n_nodes
    CL, CH, TOTC = plan.CL, plan.CH, plan.TOTC
    low_lim = plan.low_lim
    kc0 = kcs[0]

    nc = bacc.Bacc(
        "TRN2",
        target_bir_lowering=False,
        debug=False,
        num_devices=n_cores,
    )

    xT_d = nc.dram_tensor("xT", [kc0, 128, nfree], f16, kind="ExternalInput").ap()
    w_d = {}
    for l in range(3):
        kd = [d_in, d_hid, d_hid][l]
        w_d[(l, "l")] = nc.dram_tensor(f"wl{l}", [kd, douts[l]], f16,
                                       kind="ExternalInput").ap()
        w_d[(l, "r")] = nc.dram_tensor(f"wr{l}", [kd, douts[l]], f16,
                                       kind="ExternalInput").ap()
    b_d = [nc.dram_tensor(f"b{l}", [1, douts[l]], f16, kind="ExternalInput").ap()
           for l in range(3)]
    idxL_d = nc.dram_tensor("idxL", [128, max(CL * 8, 1)], i16,
                            kind="ExternalInput").ap()
    idxH_d = nc.dram_tensor("idxH", [128, max(CH * 8, 1)], i16,
                            kind="ExternalInput").ap()
    dstloc_d = nc.dram_tensor("dstloc", [128, TOTC], f16, kind="ExternalInput").ap()
    deginv_d = nc.dram_tensor("deginv", [128, nfree], f16, kind="ExternalInput").ap()
    out_d = nc.dram_tensor("out", [nsh, d_out], f32, kind="ExternalOutput").ap()

    with tile.TileContext(nc) as tc:
        # --- DRAM bounce buffers for the AllGathers
        ag_in, ag_out = [], []
        for l in range(3):
            tdt = f16 if l < 2 else f32
            ai = nc.dram_tensor(f"agi{l}", [nsh, douts[l]], tdt,
                                kind="Internal").ap()
            ao = nc.dram_tensor(f"ago{l}", [n_nodes, douts[l]], tdt,
                                kind="Internal", addr_space="Shared").ap()
            ag_in.append(ai)
            ag_out.append(ao)

        import contextlib
        with contextlib.ExitStack() as ctx:
            nb = 1 if "serial" in dbg else 3
            cpool = ctx.enter_context(tc.tile_pool(name="const", bufs=1))
            pt_pool = ctx.enter_context(
                tc.tile_pool(name="pt", bufs=2, space="PSUM"))
            pm_pool = ctx.enter_context(
                tc.tile_pool(name="pm", bufs=min(nb, 3), space="PSUM"))
            tr_pool = ctx.enter_context(
                tc.tile_pool(name="tr", bufs=2, space="PSUM"))
            tsb_pool = ctx.enter_context(tc.tile_pool(name="tsb", bufs=nb))
            msgL_pool = ctx.enter_context(tc.tile_pool(name="msgL", bufs=2))
            msgH_pool = ctx.enter_context(tc.tile_pool(name="msgH", bufs=2))
            m16_pool = ctx.enter_context(tc.tile_pool(name="m16", bufs=2))
            s_pool = ctx.enter_context(tc.tile_pool(name="spool", bufs=2))
            h_pool = ctx.enter_context(tc.tile_pool(name="hpool", bufs=2))
            o_pool = ctx.enter_context(tc.tile_pool(name="opool", bufs=2))

            # --- constants
            hT = cpool.tile([128, kc0 * nfree], f16, name="hT")
            hT3 = hT[:].rearrange("p (q n) -> p q n", n=nfree)
            nc.sync.dma_start(hT3, xT_d.rearrange("q p n -> p q n"))
            if "noping" not in dbg:
                hTb = cpool.tile([128, kc0 * nfree], f16, name="hTb")
                hT3b = hTb[:].rearrange("p (q n) -> p q n", n=nfree)
                hts = [hT3, hT3b]
            else:
                hts = [hT3, hT3]

            ident = cpool.tile([128, 128], f16, name="ident")
            make_identity(nc, ident[:])
            iota = cpool.tile([128, 128], f16, name="iota")
            nc.gpsimd.iota(iota[:], pattern=[[1, 128]], base=0,
                           channel_multiplier=0,
                           allow_small_or_imprecise_dtypes=True)
            ones = cpool.tile([1, 128], f16, name="ones")
            nc.vector.memset(ones[:], 1.0)

            wt = {}
            for l in range(3):
                kd = kcs[l]
                for s in ("l", "r"):
                    t = cpool.tile([128, kd * douts[l]], f16, name=f"w{s}{l}")
                    nc.sync.dma_start(
                        t[:].rearrange("p (q d) -> p q d", d=douts[l]),
                        w_d[(l, s)].rearrange("(q p) d -> p q d", p=128))
                    wt[(l, s)] = t
            bt = []
            for l in range(3):
                t = cpool.tile([1, douts[l]], f16, name=f"bt{l}")
                nc.sync.dma_start(t[:], b_d[l][:, :])
                bt.append(t)

            idxL_t = cpool.tile([128, max(CL * 8, 1)], i16, name="idxLt")
            nc.sync.dma_start(idxL_t[:], idxL_d[:, :])
            idxH_t = cpool.tile([128, max(CH * 8, 1)], i16, name="idxHt")
            nc.sync.dma_start(idxH_t[:], idxH_d[:, :])
            dstloc_t = cpool.tile([128, TOTC], f16, name="dstloct")
            nc.sync.dma_start(dstloc_t[:], dstloc_d[:, :])
            deginv_t = cpool.tile([128, nfree], f16, name="deginvt")
            nc.sync.dma_start(deginv_t[:], deginv_d[:, :])

            rg = [list(range(n_cores))]

            for l in range(3):
                dout = douts[l]
                kc = kcs[l]
                tdt = f16 if l < 2 else f32
                hT3 = hts[l % 2]
                hT3n = hts[(l + 1) % 2]

                # ---- phase 1: t = h @ Wl -> ag_in
                for b in range(nblk):
                    bs = slice(b * 128, (b + 1) * 128)
                    rows = min(128, nsh - b * 128)
                    pt = pt_pool.tile([128, dout], f32, tag="pt")
                    for q in range(kc):
                        nc.tensor.matmul(
                            pt[:], lhsT=hT3[:, q, bs],
                            rhs=wt[(l, "l")][:, q * dout:(q + 1) * dout],
                            start=(q == 0), stop=(q == kc - 1))
                    tsb = tsb_pool.tile([128, dout], tdt, tag="tsb")
                    nc.scalar.copy(tsb[:], pt[:])
                    nc.sync.dma_start(ag_in[l][b * 128:b * 128 + rows, :],
                                      tsb[:rows, :])

                # ---- phase 2: AllGather t
                from concourse import mybir as _mb
                if "nocoll" not in dbg:
                    nc.gpsimd.collective_compute(
                        "AllGather", _mb.AluOpType.bypass, replica_groups=rg,
                        ins=[ag_in[l].opt()], outs=[ag_out[l].opt()])

                # ---- phase 3: aggregate + root + combine per block
                for b in range(nblk):
                    bs = slice(b * 128, (b + 1) * 128)
                    rows = min(128, nsh - b * 128)
                    nL, nH = plan.nL[b], plan.nH[b]
                    nT = nL + nH

                    msgL = msgH = None
                    if nL:
                        msgL = msgL_pool.tile([128, nL * dout], tdt, tag="msgL")
                        if "nogather" in dbg:
                            nc.vector.memset(msgL[:], 0.25)
                        else:
                            nc.gpsimd.dma_gather(
                                msgL[:].rearrange("p (c e) -> p c e", e=dout),
                                ag_out[l][:, :],
                                idxL_t[:, plan.offL[b] * 8:(plan.offL[b] + nL) * 8],
                                num_idxs=nL * 128, num_idxs_reg=nL * 128,
                                elem_size=dout, single_packet=False)
                    if nH:
                        msgH = msgH_pool.tile([128, nH * dout], tdt, tag="msgH")
                        if "nogather" in dbg:
                            nc.vector.memset(msgH[:], 0.25)
                        else:
                            nc.gpsimd.dma_gather(
                                msgH[:].rearrange("p (c e) -> p c e", e=dout),
                                ag_out[l][low_lim:, :],
                                idxH_t[:, plan.offH[b] * 8:(plan.offH[b] + nH) * 8],
                                num_idxs=nH * 128, num_idxs_reg=nH * 128,
                                elem_size=dout, single_packet=False)

                    if "noagg" in dbg:
                        nT = nL = nH = 0
                    if nT:
                        S = s_pool.tile([128, nT * 128], f16, tag="S")
                        S3 = S[:].rearrange("p (c d) -> p c d", d=128)
                        dl3 = (dstloc_t[:, plan.offT[b]:plan.offT[b] + nT]
                               .rearrange("p (c o) -> p c o", o=1)
                               .to_broadcast([128, nT, 128]))
                        io3 = (iota[:].rearrange("p (o d) -> p o d", o=1)
                               .to_broadcast([128, nT, 128]))
                        nc.vector.tensor_tensor(
                            out=S3, in0=dl3, in1=io3,
                            op=_mb.AluOpType.is_equal)
                        dg3 = (deginv_t[:, bs]
                               .rearrange("p (o d) -> p o d", o=1)
                               .to_broadcast([128, nT, 128]))
                        nc.vector.tensor_tensor(
                            out=S3, in0=S3, in1=dg3, op=_mb.AluOpType.mult)

                    if l == 2 and nT:
                        m16 = m16_pool.tile([128, nT * dout], f16, tag="m16")
                        if nL:
                            nc.vector.tensor_copy(m16[:, :nL * dout], msgL[:])
                        if nH:
                            nc.vector.tensor_copy(m16[:, nL * dout:], msgH[:])

                    pm = pm_pool.tile([128, dout], f32, tag="pm")
                    for q in range(kc):
                        nc.tensor.matmul(
                            pm[:], lhsT=hT3[:, q, bs],
                            rhs=wt[(l, "r")][:, q * dout:(q + 1) * dout],
                            start=(q == 0), stop=False)
                    nc.tensor.matmul(pm[:], lhsT=ones[:1, :], rhs=bt[l][:1, :],
                                     start=False, stop=(nT == 0))
                    for j in range(nT):
                        if l == 2:
                            rhs = m16[:, j * dout:(j + 1) * dout]
                        elif j < nL:
                            rhs = msgL[:, j * dout:(j + 1) * dout]
                        else:
                            rhs = msgH[:, (j - nL) * dout:(j - nL + 1) * dout]
                        nc.tensor.matmul(pm[:], lhsT=S[:, j * 128:(j + 1) * 128],
                                         rhs=rhs, start=False,
                                         stop=(j == nT - 1))

                    if l < 2:
                        hsb = h_pool.tile([128, dout], f16, tag="h")
                        nc.scalar.activation(
                            hsb[:], pm[:],
                            _mb.ActivationFunctionType.Relu)
                        if "notr" not in dbg:
                            for q in range(kc):
                                ptr = tr_pool.tile([128, 128], f16, tag="tr")
                                nc.tensor.transpose(ptr[:], hsb[:, q * 128:(q + 1) * 128],
                                                    ident[:])
                                nc.vector.tensor_copy(hT3n[:, q, bs], ptr[:])
                    else:
                        osb = o_pool.tile([128, dout], f32, tag="o")
                        nc.scalar.copy(osb[:], pm[:])
                        nc.sync.dma_start(out_d[b * 128:b * 128 + rows, :],
                                          osb[:rows, :])

    nc.compile()
    return nc


# ---------------------------------------------------------------------------
# Entry point
# ---------------------------------------------------------------------------

LAST_RESULTS = None
_CACHE = {}


def _run(x, edge_index, weights, n_nodes, n_cores, d_in, d_hid, d_out,
         low_lim, trace=False):
    global LAST_RESULTS
    from concourse.bass_utils import run_bass_kernel_spmd

    plan, per_core = preprocess(x, edge_index, n_nodes, n_cores, d_in, low_lim)
    fp = (n_nodes, d_in, d_hid, d_out, tuple(plan.nL), tuple(plan.nH))
    if fp not in _CACHE:
        _CACHE[fp] = build_program(plan, d_in, d_hid, d_out)
    nc = _CACHE[fp]

    const = {}
    for l, (Wl, Wr, b) in enumerate(weights):
        const[f"wl{l}"] = np.asarray(Wl, np.float32).astype(np.float16)
        const[f"wr{l}"] = np.asarray(Wr, np.float32).astype(np.float16)
        const[f"b{l}"] = np.asarray(b, np.float32).astype(np.float16)[None, :]

    in_maps = []
    for c in range(n_cores):
        m = dict(const)
        pc = per_core[c]
        m["xT"] = pc["xT"]
        m["idxL"] = pc["idxL"] if plan.CL else np.zeros((128, 1), np.int16)
        m["idxH"] = pc["idxH"] if plan.CH else np.zeros((128, 1), np.int16)
        m["dstloc"] = pc["dstloc"]
        m["deginv"] = pc["deginv"]
        in_maps.append(m)

    res = run_bass_kernel_spmd(nc, in_maps, core_ids=list(range(n_cores)),
                               trace=trace)
    LAST_RESULTS = res
    out = np.concatenate([res.results[c]["out"] for c in range(n_cores)], axis=0)
    return out.astype(np.float32)


def kernel(x, edge_index, relations=None, Wl0=None, Wr0=None, b0=None,
           Wl1=None, Wr1=None, b1=None, Wl2=None, Wr2=None, b2=None,
           **kw):
    x = np.asarray(x, np.float32)
    edge_index = np.asarray(edge_index)
    weights = [(Wl0, Wr0, b0), (Wl1, Wr1, b1), (Wl2, Wr2, b2)]
    import os
    trace = bool(int(os.environ.get("KERNEL_TRACE", "0")))
    return _run(x, edge_index, weights, N_NODES, N_CORES, D_IN, D_HID, D_OUT,
                LOW_LIM_FULL, trace=trace)
